# revision 8
# baseline (speedup 1.0000x reference)
"""Trainium2 Bass kernel for DCTFrequencyFilterGVA (gnn_message_passing).

Sharding: data-parallel over points (N=65536 -> 8192/core on 8 NeuronCores).
v2 layout: the gather table is built SHARDED and AllGathered, with rows
[v(256f16) | cw(256f16) | kw(16f16)] so the main pass needs a single
indirect-gather stream (16 instr/tile) instead of three. Pos gathers for
the BN gram are interleaved with the sharded table build + q/k stats so
the SWDGE queue streams from t=0. peb/pebw share one fused matmul rhs.

Exact algebraic rewrites vs the reference:
  - key/query reduced before gathering: kw = relu(BN(k)) @ R with
    R[c,g] = glw[c] * [c//16 == g]
  - pebw = h @ (Wp2 @ R); pos-MLP first layer pre-tabulated as
    cw = coord @ Wp1; h = a_p * relu(cw_g - e_own); pos-BN stats from the
    4x4 [pos|1] Gram (exact); softmax bias bw dropped; DCT->filter->IDCT
    folded into one 16x16 matrix applied block-diagonally.
"""
import sys
sys.path.insert(0, "/opt/trn_rl_repo")
import numpy as np

import concourse.bass as bass
import concourse.bacc as bacc
import concourse.tile as tile
import concourse.masks as masks
from concourse import mybir

F32 = mybir.dt.float32
F32R = mybir.dt.float32r
F16 = mybir.dt.float16
I32 = mybir.dt.int32
AL = mybir.AluOpType
AX = mybir.AxisListType
AF = mybir.ActivationFunctionType

N, S, C, G = 65536, 16, 256, 16
NCORES = 8
NSH = N // NCORES
P = 128
NT = NSH // P              # 64 point tiles / core
CH = C // 128              # 2 channel chunks
NCHK = NSH // 512          # 16 shard chunks for q/k
TW = 528                   # tab row width (f16): v 256 | cw 256 | kw 16
EPS = 1e-5

_cache = {}


def _dct_matrix(n):
    k = np.arange(n)[:, None].astype(np.float64)
    m = np.arange(n)[None, :].astype(np.float64)
    d = np.sqrt(2.0 / n) * np.cos(np.pi * k * (2 * m + 1) / (2 * n))
    d[0, :] = np.sqrt(1.0 / n)
    return d


def _freq_filter(n, lo=0.3, hi=0.7, boost=2.0, sup=0.1):
    f = np.ones(n, np.float64)
    lc, hc = int(lo * n), int(hi * n)
    f[:lc] = boost
    if lc < hc:
        cw = 0.5 * (1 + np.cos(np.linspace(0, np.pi, hc - lc)))
        f[lc:hc] = boost * cw + 1.0 * (1 - cw)
    if hc < n:
        cw = 0.5 * (1 + np.cos(np.linspace(0, np.pi, n - hc)))
        f[hc:] = 1.0 * cw + sup * (1 - cw)
    return f


def _ap_view(ap, dims, extra_offset=0):
    return bass.AP(tensor=ap.tensor, offset=ap.offset + extra_offset, ap=dims)


def build_nc(debug=False):
    nc = bacc.Bacc("TRN2", target_bir_lowering=False, debug=False,
                   num_devices=NCORES)
    DP = {}

    def dram_in(name, shape, dt=F32):
        DP[name] = nc.declare_dram_parameter(name, list(shape), dt, isOutput=False)

    dram_in("featT_sh", [C, NSH], F32R)
    dram_in("coordT_sh", [3, NSH], F32R)
    dram_in("coord_full", [N, 3], F32)
    dram_in("idxc", [NSH, S], I32)
    dram_in("idxraw_f", [NSH, S], F32)
    dram_in("Wqkv", [C, 3 * C], F32R)
    dram_in("Wp1", [3, C], F32R)
    dram_in("Wp2", [C, C], F32)
    dram_in("Wp2w", [C, G], F32)
    dram_in("Rglw", [C, G], F32)
    dram_in("WwBD", [S * G, S * G], F32)
    dram_in("Wg1", [C, C // 4], F32)
    dram_in("Wg2", [C // 4, 1], F32)
    dram_in("BDdct", [C, C], F32)
    dram_in("bias_rows", [8, C], F32)
    dram_in("qk_cm", [4 * C, 1], F32)   # [gq betaq gk betak] channel-major
    dram_in("p_cm", [3 * C, 1], F32)    # [gp betap bp1] channel-major
    dram_in("b_cm", [2 * C, 1], F32)    # [bq bk] channel-major

    out_p = nc.declare_dram_parameter("out", [NSH, C], F32, isOutput=True)

    tabA_sh = nc.dram_tensor("tabA_sh", [NSH, TW], F16)
    tabA = nc.dram_tensor("tabA", [N, TW], F16, addr_space="Shared")
    qkT_dram = nc.dram_tensor("qkT_dram", [2, C, NSH], F16)
    qw_dram = nc.dram_tensor("qw_dram", [NSH, G], F32)
    valbuf = nc.dram_tensor("valbuf", [NSH, S * C], F16)
    ar1_i = nc.dram_tensor("ar1_i", [72, 128], F32)
    ar1_o = nc.dram_tensor("ar1_o", [72, 128], F32)
    ar2_i = nc.dram_tensor("ar2_i", [1, 64], F32)
    ar2_o = nc.dram_tensor("ar2_o", [1, 64], F32)

    with tile.TileContext(nc) as tc:
        _body(nc, tc, DP, out_p, tabA_sh, tabA, qkT_dram, qw_dram, valbuf,
              ar1_i, ar1_o, ar2_i, ar2_o)
    nc.compile()
    return nc


def _body(nc, tc, DP, out_p, tabA_sh, tabA, qkT_dram, qw_dram, valbuf,
          ar1_i, ar1_o, ar2_i, ar2_o):
    from contextlib import ExitStack
    ctx = ExitStack()
    const = ctx.enter_context(tc.tile_pool(name="const", bufs=1))
    small = ctx.enter_context(tc.tile_pool(name="small", bufs=1))
    work = ctx.enter_context(tc.tile_pool(name="work", bufs=2))
    gpool = ctx.enter_context(tc.tile_pool(name="gpool", bufs=2))
    stash = ctx.enter_context(tc.tile_pool(name="stash", bufs=1))
    psA = ctx.enter_context(tc.tile_pool(name="psA", bufs=2, space="PSUM"))
    psAcc = ctx.enter_context(tc.tile_pool(name="psAcc", bufs=1, space="PSUM"))

    _uid = [0]
    def uname(p="t"):
        _uid[0] += 1
        return f"{p}{_uid[0]}"

    def cload(name, r0, r1, pool=const, dt=None):
        ap = DP[name].ap()[r0:r1, :]
        t = pool.tile([r1 - r0, ap.shape[1]], dt or DP[name].ap().dtype,
                      name=uname("c_" + name))
        nc.sync.dma_start(out=t[:], in_=ap)
        return t

    def to_f16(t, shape=None):
        o = const.tile(shape or list(t.shape), F16, name=uname("f16"))
        nc.vector.tensor_copy(o[:], t[:])
        return o

    def brow(row, n):
        ap = DP["bias_rows"].ap()
        t = const.tile([P, n], F32, name=uname("brow"))
        nc.gpsimd.dma_start(out=t[:], in_=_ap_view(ap, [[0, P], [1, n]], row * C))
        return t

    # ------------------------- constants -------------------------
    Wqkv = [cload("Wqkv", k * 128, (k + 1) * 128) for k in range(CH)]
    Wp1 = cload("Wp1", 0, 3)
    Wp2 = [cload("Wp2", k * 128, (k + 1) * 128) for k in range(CH)]
    Wp2w = [cload("Wp2w", k * 128, (k + 1) * 128) for k in range(CH)]
    Rg16 = [to_f16(cload("Rglw", k * 128, (k + 1) * 128, pool=const)) for k in range(CH)]
    WwBD16 = [to_f16(cload("WwBD", k * 128, (k + 1) * 128, pool=const))
              for k in range(CH)]
    Wg1_16 = [to_f16(cload("Wg1", k * 128, (k + 1) * 128, pool=const)) for k in range(CH)]
    Wg2_16 = to_f16(cload("Wg2", 0, C // 4, pool=const))
    BD16 = [to_f16(cload("BDdct", k * 128, (k + 1) * 128, pool=const)) for k in range(CH)]

    bvp2_r = brow(0, C)
    bg1_r = brow(2, C // 4)
    bg2_r = brow(3, 1)
    bp2w_r = brow(4, G)
    gw_r = brow(5, G)
    betaw_r = brow(6, G)

    qk_cm = [cload("qk_cm", j * 128, (j + 1) * 128) for j in range(4 * CH)]
    p_cm = [cload("p_cm", j * 128, (j + 1) * 128) for j in range(3 * CH)]
    b_cm = [cload("b_cm", j * 128, (j + 1) * 128) for j in range(2 * CH)]

    ident32 = const.tile([P, P], F32, name="anon1")
    masks.make_identity(nc, ident32[:])
    ident16 = const.tile([P, P], F16, name="anon2")
    nc.vector.tensor_copy(ident16[:], ident32[:])
    ones16 = const.tile([P, 1], F16, name="anon3")
    nc.vector.memset(ones16[:], 1.0)
    ones3 = const.tile([3, 1], F32, name="ones3")
    nc.vector.memset(ones3[:], 1.0)
    eps_col = const.tile([P, 1], F32, name="eps_col")
    nc.vector.memset(eps_col[:], EPS)
    neg1 = const.tile([P, 1], F32, name="neg1")
    nc.vector.memset(neg1[:], -1.0)
    nbg2 = const.tile([P, 1], F32, name="nbg2")
    nc.vector.tensor_scalar(out=nbg2[:], in0=bg2_r[:, 0:1], scalar1=-1.0,
                            scalar2=None, op0=AL.mult)

    # index / mask loads (needed by the pos gathers immediately)
    idx_all = stash.tile([P, NT * S], I32, tag="idx_all", name="idx_all")
    nc.sync.dma_start(out=idx_all[:],
                      in_=_ap_view(DP["idxc"].ap(), [[S, P], [P * S, NT], [1, S]]))
    raw_all = work.tile([P, NT * S], F32, tag="raw_all", name="raw_all")
    nc.sync.dma_start(out=raw_all[:],
                      in_=_ap_view(DP["idxraw_f"].ap(), [[S, P], [P * S, NT], [1, S]]))
    mask_all = stash.tile([P, NT * S], F16, tag="mask_all", name="mask_all")
    nc.vector.tensor_scalar(out=mask_all[:], in0=raw_all[:], scalar1=0.0,
                            scalar2=None, op0=AL.is_ge)

    # =============================================================
    # Stage 1 (interleaved per tile): pos gather + [pos|1] gram,
    # sharded tab build (v|cw), q/k channel-major + stats
    # =============================================================
    st_acc = [[stash.tile([128, 2 * NCHK], F32, tag=f"st{w}{m}", name="anon4") for m in range(CH)]
              for w in range(2)]
    pgram = psAcc.tile([16, 16], F32, tag="accA", name="accA")

    for t in range(NT):
        r0 = t * P
        # --- pos gather + gram increment ---
        po = work.tile([P, 4, 4], F32, tag="p3po", name="p3po")
        for si, s in enumerate(range(0, S, 4)):
            nc.gpsimd.indirect_dma_start(
                out=po[:, si, 0:3], out_offset=None,
                in_=DP["coord_full"].ap(),
                in_offset=bass.IndirectOffsetOnAxis(
                    ap=idx_all[:, t * S + s:t * S + s + 1], axis=0))
        cot = work.tile([3, P], F32, tag="p3cot", name="p3cot")
        nc.sync.dma_start(out=cot[:],
                          in_=DP["coordT_sh"].ap()[:, r0:r0 + P].bitcast(F32))
        pco = psA.tile([P, 3], F32, tag="mm", name="mm")
        nc.tensor.matmul(pco[:], cot[:], ident32[0:3, 0:3], is_transpose=True)
        coP = work.tile([P, 3], F32, tag="p3co", name="p3co")
        nc.scalar.copy(coP[:], pco[:])
        nc.vector.tensor_tensor(
            out=po[:, :, 0:3], in0=po[:, :, 0:3],
            in1=bass.AP(tensor=coP.tensor, offset=coP.offset,
                        ap=[coP.ap[0], [0, 4], [1, 3]]),
            op=AL.subtract)
        nc.vector.memset(po[:, :, 3:4], 1.0)
        pof = po[:].rearrange("p s d -> p (s d)")
        nc.tensor.matmul(pgram[:], pof, pof, start=(t == 0), stop=(t == NT - 1))

        # --- sharded tab build: v and cw for this tile ---
        ft = [work.tile([128, P], F32R, tag="p1ft", name="p1ft") for _ in range(CH)]
        for k in range(CH):
            nc.sync.dma_start(out=ft[k][:],
                              in_=DP["featT_sh"].ap()[k * 128:(k + 1) * 128, r0:r0 + P])
        pv = psA.tile([P, C], F32, tag="mm", name="mm")
        for k in range(CH):
            nc.tensor.matmul(pv[:], ft[k][:], Wqkv[k][:, 2 * C:3 * C].bitcast(F32R),
                             start=(k == 0), stop=(k == CH - 1))
        vsb = work.tile([P, C], F16, tag="p1v", name="p1v")
        nc.vector.tensor_tensor(out=vsb[:], in0=pv[:], in1=bvp2_r[:], op=AL.add)
        nc.sync.dma_start(
            out=_ap_view(tabA_sh.ap(), [[TW, P], [1, C]], r0 * TW),
            in_=vsb[:])
        ct2 = work.tile([3, P], F32R, tag="p1ct", name="p1ct")
        nc.sync.dma_start(out=ct2[:], in_=DP["coordT_sh"].ap()[:, r0:r0 + P])
        pc = psA.tile([P, C], F32, tag="mm", name="mm")
        nc.tensor.matmul(pc[:], ct2[:], Wp1[:], start=True, stop=True)
        csb = work.tile([P, C], F16, tag="p1c", name="p1c")
        nc.scalar.copy(csb[:], pc[:])
        nc.sync.dma_start(
            out=_ap_view(tabA_sh.ap(), [[TW, P], [1, C]], r0 * TW + C),
            in_=csb[:])

        # --- q/k channel-major + per-channel sum/sumsq (every 4th tile) ---
        if t % 4 == 3:
            j = t // 4
            p0 = j * 512
            fs = [work.tile([128, 512], F32R, tag="p2fs", name="p2fs") for _ in range(CH)]
            for k in range(CH):
                nc.sync.dma_start(out=fs[k][:],
                                  in_=DP["featT_sh"].ap()[k * 128:(k + 1) * 128, p0:p0 + 512])
            for w in range(2):
                for m in range(CH):
                    pq = psA.tile([128, 512], F32, tag="mm", name="mm")
                    for k in range(CH):
                        nc.tensor.matmul(
                            pq[:],
                            Wqkv[k][:, w * C + m * 128:w * C + (m + 1) * 128].bitcast(F32R),
                            fs[k][:], start=(k == 0), stop=(k == CH - 1))
                    qc = work.tile([128, 512], F16, tag="p2qc", name="p2qc")
                    nc.scalar.copy(qc[:], pq[:])
                    sq = work.tile([128, 512], F32, tag="p2sq", name="p2sq")
                    nc.vector.tensor_tensor(out=sq[:], in0=qc[:], in1=qc[:], op=AL.mult)
                    nc.vector.tensor_reduce(out=st_acc[w][m][:, 2 * j:2 * j + 1],
                                            in_=pq[:], axis=AX.X, op=AL.add)
                    nc.vector.tensor_reduce(out=st_acc[w][m][:, 2 * j + 1:2 * j + 2],
                                            in_=sq[:], axis=AX.X, op=AL.add)
                    nc.sync.dma_start(
                        out=qkT_dram.ap()[w, m * 128:(m + 1) * 128, p0:p0 + 512],
                        in_=qc[:])

    stat2 = [[small.tile([128, 2], F32, tag=f"s2_{w}{m}", name="anon5") for m in range(CH)]
             for w in range(2)]
    for w in range(2):
        for m in range(CH):
            v = st_acc[w][m][:]
            vv = bass.AP(tensor=v.tensor, offset=v.offset,
                         ap=[v.ap[0], [1, 2], [2, NCHK]])
            nc.vector.tensor_reduce(out=stat2[w][m][:], in_=vv, axis=AX.X, op=AL.add)

    # =============================================================
    # AllReduce #1 + affine derivation
    # =============================================================
    for w in range(2):
        for m in range(CH):
            base = (64 + 2 * (2 * w + m)) * 128
            nc.sync.dma_start(
                out=_ap_view(ar1_i.ap(), [[2, 128], [1, 2]], base),
                in_=stat2[w][m][:])
    gr_sb = small.tile([16, 16], F32, tag="gr_sb", name="gr_sb")
    nc.vector.tensor_copy(gr_sb[:], pgram[:])
    nc.sync.dma_start(out=_ap_view(ar1_i.ap(), [[128, 16], [1, 16]], 0),
                      in_=gr_sb[:])
    nc.gpsimd.collective_compute(
        "AllReduce", AL.add, replica_groups=[list(range(NCORES))],
        ins=[ar1_i.ap().opt()], outs=[ar1_o.ap().opt()])

    stat2g = [[small.tile([128, 2], F32, tag=f"s2g{w}{m}", name="anon6") for m in range(CH)]
              for w in range(2)]
    for w in range(2):
        for m in range(CH):
            base = (64 + 2 * (2 * w + m)) * 128
            nc.sync.dma_start(
                out=stat2g[w][m][:],
                in_=_ap_view(ar1_o.ap(), [[2, 128], [1, 2]], base))
    grg = small.tile([16, 16], F32, tag="grg", name="grg")
    nc.sync.dma_start(out=grg[:],
                      in_=_ap_view(ar1_o.ap(), [[128, 16], [1, 16]], 0))

    def bn_affine(sums, cnt, gamma, beta, bias):
        mean = small.tile([128, 1], F32, name=uname("sm7_"))
        nc.vector.tensor_scalar(out=mean[:], in0=sums[:, 0:1],
                                scalar1=1.0 / cnt, scalar2=None, op0=AL.mult)
        ex2 = small.tile([128, 1], F32, name=uname("sm8_"))
        nc.vector.tensor_scalar(out=ex2[:], in0=sums[:, 1:2],
                                scalar1=1.0 / cnt, scalar2=None, op0=AL.mult)
        var = small.tile([128, 1], F32, name=uname("sm9_"))
        nc.vector.tensor_tensor(out=var[:], in0=mean[:], in1=mean[:], op=AL.mult)
        nc.vector.tensor_tensor(out=var[:], in0=ex2[:], in1=var[:], op=AL.subtract)
        meanb = small.tile([128, 1], F32, name=uname("sm10_"))
        if bias is not None:
            nc.vector.tensor_tensor(out=meanb[:], in0=mean[:], in1=bias[:], op=AL.add)
        else:
            nc.vector.tensor_copy(meanb[:], mean[:])
        sd = small.tile([128, 1], F32, name=uname("sm11_"))
        nc.scalar.activation(out=sd[:], in_=var[:], func=AF.Sqrt, bias=eps_col[0:var.shape[0], :])
        rsd = small.tile([128, 1], F32, name=uname("sm12_"))
        nc.vector.reciprocal(out=rsd[:], in_=sd[:])
        a = small.tile([128, 1], F32, name=uname("sm13_"))
        nc.vector.tensor_tensor(out=a[:], in0=gamma[:], in1=rsd[:], op=AL.mult)
        d = small.tile([128, 1], F32, name=uname("sm14_"))
        nc.vector.tensor_tensor(out=d[:], in0=a[:], in1=meanb[:], op=AL.mult)
        nc.vector.tensor_tensor(out=d[:], in0=beta[:], in1=d[:], op=AL.subtract)
        return a, d

    qk_aff = []
    for w in range(2):
        for m in range(CH):
            qk_aff.append(bn_affine(stat2g[w][m], float(N),
                                    qk_cm[2 * w * CH + m],
                                    qk_cm[(2 * w + 1) * CH + m],
                                    b_cm[w * CH + m]))

    s4p = psA.tile([4, 4], F32, tag="mm", name="mm")
    for s in range(4):
        nc.tensor.matmul(s4p[:], ident32[0:16, 4 * s:4 * s + 4],
                         grg[0:16, 4 * s:4 * s + 4], start=(s == 0), stop=(s == 3))
    s4 = small.tile([4, 4], F32, tag="s4", name="s4")
    nc.vector.tensor_scalar(out=s4[:], in0=s4p[:], scalar1=1.0 / (N * 4),
                            scalar2=None, op0=AL.mult)
    Tmp = psA.tile([3, C], F32, tag="mm", name="mm")
    nc.tensor.matmul(Tmp[:], s4[0:3, 0:3], Wp1[:].bitcast(F32), start=True, stop=True)
    WT = small.tile([3, C], F32, tag="WT", name="WT")
    nc.vector.tensor_tensor(out=WT[:], in0=Tmp[:], in1=Wp1[:].bitcast(F32), op=AL.mult)

    ap_cm, ebase_cm = [], []
    for m in range(CH):
        mh = psA.tile([128, 1], F32, tag="mm", name="mm")
        nc.tensor.matmul(mh[:], Wp1[:, m * 128:(m + 1) * 128].bitcast(F32),
                         s4[0:3, 3:4], start=True, stop=True)
        mh_sb = small.tile([128, 1], F32, name=uname("sm15_"))
        nc.vector.tensor_copy(mh_sb[:], mh[:])
        e2 = psA.tile([128, 1], F32, tag="mm", name="mm")
        nc.tensor.matmul(e2[:], WT[:, m * 128:(m + 1) * 128], ones3[:],
                         start=True, stop=True)
        var = small.tile([128, 1], F32, name=uname("sm16_"))
        nc.vector.tensor_tensor(out=var[:], in0=mh_sb[:], in1=mh_sb[:], op=AL.mult)
        nc.vector.tensor_tensor(out=var[:], in0=e2[:], in1=var[:], op=AL.subtract)
        sd = small.tile([128, 1], F32, name=uname("sm17_"))
        nc.scalar.activation(out=sd[:], in_=var[:], func=AF.Sqrt, bias=eps_col[0:var.shape[0], :])
        rsd = small.tile([128, 1], F32, name=uname("sm18_"))
        nc.vector.reciprocal(out=rsd[:], in_=sd[:])
        a = small.tile([128, 1], F32, name=uname("sm19_"))
        nc.vector.tensor_tensor(out=a[:], in0=p_cm[m][:], in1=rsd[:], op=AL.mult)
        ap_cm.append(a)
        eb = small.tile([128, 1], F32, name=uname("sm20_"))
        nc.vector.tensor_tensor(out=eb[:], in0=p_cm[CH + m][:], in1=sd[:], op=AL.mult)
        gprec = small.tile([128, 1], F32, name=uname("sm21_"))
        nc.vector.reciprocal(out=gprec[:], in_=p_cm[m][:])
        nc.vector.tensor_tensor(out=eb[:], in0=eb[:], in1=gprec[:], op=AL.mult)
        nc.vector.tensor_tensor(out=eb[:], in0=eb[:], in1=mh_sb[:], op=AL.subtract)
        ebase_cm.append(eb)

    # fold a_p into fused [Wp2 | Wp2w] (f16)
    Wp2pm16 = []
    for m in range(CH):
        t1 = const.tile([128, C + G], F16, name=uname("Wp2pm16_"))
        nc.vector.tensor_scalar(out=t1[:, 0:C], in0=Wp2[m][:], scalar1=ap_cm[m][:],
                                scalar2=None, op0=AL.mult)
        nc.vector.tensor_scalar(out=t1[:, C:C + G], in0=Wp2w[m][:], scalar1=ap_cm[m][:],
                                scalar2=None, op0=AL.mult)
        Wp2pm16.append(t1)

    ebrow = small.tile([1, C], F32, tag="ebrow", name="ebrow")
    for m in range(CH):
        pt = psA.tile([1, 128], F32, tag="mm", name="mm")
        nc.tensor.matmul(pt[:], ebase_cm[m][:], ident32[0:128, 0:128],
                         is_transpose=True)
        nc.vector.tensor_copy(ebrow[:, m * 128:(m + 1) * 128], pt[:])
    ebase_rep = const.tile([P, C], F32, name="anon24")
    nc.gpsimd.partition_broadcast(ebase_rep[:], ebrow[:])

    # =============================================================
    # Phase 5: q/k finalize; qw -> DRAM, kw (f16) -> tabA_sh cols; AG
    # =============================================================
    for j in range(NCHK):
        p0 = j * 512
        for w in range(2):
            kr = [work.tile([128, 512], F16, tag="p5kr", name="p5kr") for _ in range(CH)]
            for m in range(CH):
                a, d = qk_aff[w * CH + m]
                qt = work.tile([128, 512], F16, tag="p5qt", name="p5qt")
                nc.sync.dma_start(
                    out=qt[:],
                    in_=qkT_dram.ap()[w, m * 128:(m + 1) * 128, p0:p0 + 512])
                nc.scalar.activation(out=kr[m][:], in_=qt[:],
                                     func=AF.Relu, bias=d[:], scale=a[:])
            pk = psA.tile([G, 512], F32, tag="mm", name="mm")
            for m in range(CH):
                nc.tensor.matmul(pk[:], Rg16[m][:], kr[m][:],
                                 start=(m == 0), stop=(m == CH - 1))
            kT_sb = work.tile([G, 512], F32, tag="p5kT", name="p5kT")
            nc.scalar.copy(kT_sb[:], pk[:])
            pkP = psA.tile([128, 4, G], F32, tag="mm", name="mm")
            for sub in range(4):
                nc.tensor.matmul(pkP[:, sub, :], kT_sb[:, sub * 128:(sub + 1) * 128],
                                 ident32[0:G, 0:G], is_transpose=True)
            if w == 0:
                kP = work.tile([128, 4, G], F32, tag="p5kP", name="p5kP")
                nc.vector.tensor_tensor(
                    out=kP[:], in0=pkP[:],
                    in1=bass.AP(tensor=bp2w_r.tensor, offset=bp2w_r.offset,
                                ap=[bp2w_r.ap[0], [0, 4], [1, G]]),
                    op=AL.subtract)
                nc.sync.dma_start(
                    out=_ap_view(qw_dram.ap(), [[G, 128], [128 * G, 4], [1, G]],
                                 p0 * G),
                    in_=kP[:])
            else:
                kf = work.tile([128, 4, G], F16, tag="p5kf", name="p5kf")
                nc.vector.tensor_copy(kf[:], pkP[:])
                nc.sync.dma_start(
                    out=_ap_view(tabA_sh.ap(), [[TW, 128], [128 * TW, 4], [1, G]],
                                 p0 * TW + 2 * C),
                    in_=kf[:])
    nc.gpsimd.collective_compute(
        "AllGather", AL.bypass, replica_groups=[list(range(NCORES))],
        ins=[tabA_sh.ap().opt()], outs=[tabA.ap().opt()])

    # =============================================================
    # Main pass 1: single gather stream, h/peb/pebw, val stash, w stats
    # =============================================================
    wpre_all = stash.tile([P, NT, S * G], F16, tag="wpre_all", name="wpre_all")
    ps_ws = psAcc.tile([1, 2 * S * G], F32, tag="ps_ws", name="ps_ws")

    for t in range(NT):
        r0 = t * P
        gt = gpool.tile([P, S, TW], F16, tag="gt", name="gt")
        for s in range(S):
            nc.gpsimd.indirect_dma_start(
                out=gt[:, s, :], out_offset=None, in_=tabA.ap(),
                in_offset=bass.IndirectOffsetOnAxis(
                    ap=idx_all[:, t * S + s:t * S + s + 1], axis=0))
        cot = work.tile([3, P], F32R, tag="m1cot", name="m1cot")
        nc.sync.dma_start(out=cot[:], in_=DP["coordT_sh"].ap()[:, r0:r0 + P])
        pcw = psA.tile([P, C], F32, tag="mm", name="mm")
        nc.tensor.matmul(pcw[:], cot[:], Wp1[:], start=True, stop=True)
        e_own = work.tile([P, C], F16, tag="m1eo", name="m1eo")
        nc.vector.tensor_tensor(out=e_own[:], in0=pcw[:], in1=ebase_rep[:],
                                op=AL.subtract)
        qwt = work.tile([P, G], F32, tag="m1qw", name="m1qw")
        nc.sync.dma_start(out=qwt[:], in_=qw_dram.ap()[r0:r0 + P, :])

        # h for all s at once: relu(cw_g - e_own)
        h_all = work.tile([P, S, C], F16, tag="m1h", name="m1h")
        nc.vector.tensor_tensor(
            out=h_all[:],
            in0=bass.AP(tensor=gt.tensor, offset=gt.offset + C,
                        ap=[gt.ap[0], [TW, S], [1, C]]),
            in1=bass.AP(tensor=e_own.tensor, offset=e_own.offset,
                        ap=[e_own.ap[0], [0, S], [1, C]]),
            op=AL.subtract)
        nc.scalar.activation(out=h_all[:], in_=h_all[:], func=AF.Relu)

        val = gpool.tile([P, S, C], F16, tag="val", name="val")
        pebw_sb = work.tile([P, S, G], F32, tag="m1pw", name="m1pw")
        for s in range(S):
            ps_hT = psA.tile([P, CH, P], F16, tag="hT", name="hT")
            for m in range(CH):
                nc.tensor.matmul(ps_hT[:, m, :], h_all[:, s, m * 128:(m + 1) * 128],
                                 ident16[:], is_transpose=True)
            hT = work.tile([P, CH, P], F16, tag="m1hT", name="m1hT")
            nc.scalar.copy(hT[:], ps_hT[:])
            ps_pebm = psA.tile([P, C + G], F32, tag="pebm", name="pebm")
            for m in range(CH):
                nc.tensor.matmul(ps_pebm[:], hT[:, m, :], Wp2pm16[m][:],
                                 start=(m == 0), stop=(m == CH - 1))
            nc.vector.tensor_tensor(out=val[:, s, :], in0=gt[:, s, 0:C],
                                    in1=ps_pebm[:, 0:C], op=AL.add)
            nc.vector.tensor_copy(pebw_sb[:, s, :], ps_pebm[:, C:C + G])
        nc.sync.dma_start(out=valbuf.ap()[r0:r0 + P, :],
                          in_=val[:].rearrange("p s c -> p (s c)"))

        wq = work.tile([P, S, G], F32, tag="m1wq", name="m1wq")
        nc.vector.tensor_tensor(
            out=wq[:],
            in0=bass.AP(tensor=gt.tensor, offset=gt.offset + 2 * C,
                        ap=[gt.ap[0], [TW, S], [1, G]]),
            in1=bass.AP(tensor=qwt.tensor, offset=qwt.offset,
                        ap=[qwt.ap[0], [0, S], [1, G]]),
            op=AL.subtract)
        nc.vector.tensor_tensor(out=wpre_all[:, t, :],
                                in0=wq[:].rearrange("p s g -> p (s g)"),
                                in1=pebw_sb[:].rearrange("p s g -> p (s g)"),
                                op=AL.add)
        sqw = work.tile([P, S * G], F16, tag="m1sqw", name="m1sqw")
        nc.vector.tensor_tensor(out=sqw[:], in0=wpre_all[:, t, :],
                                in1=wpre_all[:, t, :], op=AL.mult)
        nc.tensor.matmul(ps_ws[:, 0:S * G], ones16[:], wpre_all[:, t, :],
                         start=(t == 0), stop=(t == NT - 1))
        nc.tensor.matmul(ps_ws[:, S * G:2 * S * G], ones16[:], sqw[:],
                         start=(t == 0), stop=(t == NT - 1))

    # =============================================================
    # AllReduce #2: w stats -> aw/dw rows, replicated
    # =============================================================
    wstat = small.tile([1, 64], F32, tag="wstat", name="wstat")
    nc.vector.tensor_reduce(
        out=wstat[:, 0:G],
        in_=bass.AP(tensor=ps_ws.tensor, offset=ps_ws.offset,
                    ap=[ps_ws.ap[0], [1, G], [G, S]]),
        axis=AX.X, op=AL.add)
    nc.vector.tensor_reduce(
        out=wstat[:, G:2 * G],
        in_=bass.AP(tensor=ps_ws.tensor, offset=ps_ws.offset + S * G,
                    ap=[ps_ws.ap[0], [1, G], [G, S]]),
        axis=AX.X, op=AL.add)
    nc.sync.dma_start(out=ar2_i.ap()[:, :], in_=wstat[:])
    nc.gpsimd.collective_compute(
        "AllReduce", AL.add, replica_groups=[list(range(NCORES))],
        ins=[ar2_i.ap().opt()], outs=[ar2_o.ap().opt()])
    wstatg = small.tile([1, 64], F32, tag="wstatg", name="wstatg")
    nc.sync.dma_start(out=wstatg[:], in_=ar2_o.ap()[:, :])

    mw = small.tile([1, G], F32, name=uname("sg25_"))
    nc.vector.tensor_scalar(out=mw[:], in0=wstatg[:, 0:G], scalar1=1.0 / (N * S),
                            scalar2=None, op0=AL.mult)
    vw = small.tile([1, G], F32, name=uname("sg26_"))
    nc.vector.tensor_scalar(out=vw[:], in0=wstatg[:, G:2 * G], scalar1=1.0 / (N * S),
                            scalar2=None, op0=AL.mult)
    m2w = small.tile([1, G], F32, name=uname("sg27_"))
    nc.vector.tensor_tensor(out=m2w[:], in0=mw[:], in1=mw[:], op=AL.mult)
    nc.vector.tensor_tensor(out=vw[:], in0=vw[:], in1=m2w[:], op=AL.subtract)
    sdw = small.tile([1, G], F32, name=uname("sg28_"))
    nc.scalar.activation(out=sdw[:], in_=vw[:], func=AF.Sqrt, bias=eps_col[0:1, :])
    rsdw = small.tile([1, G], F32, name=uname("sg29_"))
    nc.vector.reciprocal(out=rsdw[:], in_=sdw[:])
    awr = small.tile([1, G], F32, name=uname("sg30_"))
    nc.vector.tensor_tensor(out=awr[:], in0=gw_r[0:1, :], in1=rsdw[:], op=AL.mult)
    dwr = small.tile([1, G], F32, name=uname("sg31_"))
    nc.vector.tensor_tensor(out=dwr[:], in0=awr[:], in1=mw[:], op=AL.mult)
    nc.vector.tensor_tensor(out=dwr[:], in0=betaw_r[0:1, :], in1=dwr[:], op=AL.subtract)
    aw_rep = const.tile([P, G], F16, name="anon32")
    dw_rep = const.tile([P, G], F16, name="anon33")
    awf = small.tile([1, G], F16, name="anon34")
    dwf = small.tile([1, G], F16, name="anon35")
    nc.vector.tensor_copy(awf[:], awr[:])
    nc.vector.tensor_copy(dwf[:], dwr[:])
    nc.gpsimd.partition_broadcast(aw_rep[:], awf[:])
    nc.gpsimd.partition_broadcast(dw_rep[:], dwf[:])

    # =============================================================
    # Main pass 2: BN(w)+relu, w@Ww, softmax+mask, aggregate, DCT, gate
    # =============================================================
    for t in range(NT):
        r0 = t * P
        wp = wpre_all[:, t, :]
        w2 = work.tile([P, S * G], F16, tag="m2w2", name="m2w2")
        nc.vector.tensor_tensor(
            out=w2[:], in0=wp,
            in1=bass.AP(tensor=aw_rep.tensor, offset=aw_rep.offset,
                        ap=[aw_rep.ap[0], [0, S], [1, G]]),
            op=AL.mult)
        nc.vector.tensor_tensor(
            out=w2[:], in0=w2[:],
            in1=bass.AP(tensor=dw_rep.tensor, offset=dw_rep.offset,
                        ap=[dw_rep.ap[0], [0, S], [1, G]]),
            op=AL.add)
        nc.vector.tensor_scalar(out=w2[:], in0=w2[:], scalar1=0.0, scalar2=None,
                                op0=AL.max)
        ps_w2T = psA.tile([P, CH, P], F16, tag="hT", name="ps_w2T")
        for m in range(CH):
            nc.tensor.matmul(ps_w2T[:, m, :], w2[:, m * 128:(m + 1) * 128],
                             ident16[:], is_transpose=True)
        w2T = work.tile([P, CH, P], F16, tag="m2w2T", name="w2T")
        nc.scalar.copy(w2T[:], ps_w2T[:])
        ps_w3 = psAcc.tile([P, S, G], F32, tag="accA", name="ps_w3")
        w3flat = ps_w3[:].rearrange("p s g -> p (s g)")
        for k in range(CH):
            nc.tensor.matmul(w3flat, w2T[:, k, :], WwBD16[k][:],
                             start=(k == 0), stop=(k == CH - 1))
        mx = work.tile([P, G], F32, tag="m2mx", name="m2mx")
        nc.vector.tensor_reduce(
            out=mx[:],
            in_=bass.AP(tensor=ps_w3.tensor, offset=ps_w3.offset,
                        ap=[ps_w3.ap[0], [1, G], [G, S]]),
            axis=AX.X, op=AL.max)
        eb = work.tile([P, S, G], F16, tag="m2eb", name="m2eb")
        nc.vector.tensor_tensor(
            out=eb[:], in0=ps_w3[:],
            in1=bass.AP(tensor=mx.tensor, offset=mx.offset,
                        ap=[mx.ap[0], [0, S], [1, G]]),
            op=AL.subtract)
        nc.scalar.activation(out=eb[:], in_=eb[:], func=AF.Exp)
        dn = work.tile([P, G], F32, tag="m2dn", name="m2dn")
        nc.vector.tensor_reduce(
            out=dn[:],
            in_=bass.AP(tensor=eb.tensor, offset=eb.offset,
                        ap=[eb.ap[0], [1, G], [G, S]]),
            axis=AX.X, op=AL.add)
        rdn = work.tile([P, G], F32, tag="m2rdn", name="m2rdn")
        nc.vector.reciprocal(out=rdn[:], in_=dn[:])
        wf = work.tile([P, S, G], F16, tag="m2wf", name="m2wf")
        nc.vector.tensor_tensor(
            out=wf[:], in0=eb[:],
            in1=bass.AP(tensor=rdn.tensor, offset=rdn.offset,
                        ap=[rdn.ap[0], [0, S], [1, G]]),
            op=AL.mult)
        nc.vector.tensor_tensor(
            out=wf[:], in0=wf[:],
            in1=bass.AP(tensor=mask_all.tensor,
                        offset=mask_all.offset + t * S,
                        ap=[mask_all.ap[0], [1, S], [0, G]]),
            op=AL.mult)

        val = gpool.tile([P, S, C], F16, tag="val", name="val2")
        nc.sync.dma_start(out=val[:].rearrange("p s c -> p (s c)"),
                          in_=valbuf.ap()[r0:r0 + P, :])
        u = val
        nc.vector.tensor_tensor(
            out=u[:], in0=val[:],
            in1=bass.AP(tensor=wf.tensor, offset=wf.offset,
                        ap=[wf.ap[0], [G, S], [1, G], [0, G]]),
            op=AL.mult)
        spat = work.tile([P, C], F32, tag="m2sp", name="m2sp")
        nc.vector.tensor_reduce(
            out=spat[:],
            in_=bass.AP(tensor=u.tensor, offset=u.offset,
                        ap=[u.ap[0], [1, C], [C, S]]),
            axis=AX.X, op=AL.add)

        ps_spT = psA.tile([P, CH, P], F32, tag="mm", name="mm")
        for m in range(CH):
            nc.tensor.matmul(ps_spT[:, m, :], spat[:, m * 128:(m + 1) * 128],
                             ident32[:], is_transpose=True)
        spT = work.tile([P, CH, P], F16, tag="m2spT", name="m2spT")
        nc.scalar.copy(spT[:], ps_spT[:])
        ps_enT = psA.tile([P, CH, P], F32, tag="mm", name="mm")
        for mo in range(CH):
            for k in range(CH):
                nc.tensor.matmul(ps_enT[:, mo, :],
                                 BD16[k][:, mo * 128:(mo + 1) * 128],
                                 spT[:, k, :], start=(k == 0), stop=(k == CH - 1))
        enT = work.tile([P, CH, P], F16, tag="m2enT", name="m2enT")
        nc.scalar.copy(enT[:], ps_enT[:])
        ps_en = psA.tile([P, C], F16, tag="hT", name="hT")
        for m in range(CH):
            nc.tensor.matmul(ps_en[:, m * 128:(m + 1) * 128], enT[:, m, :],
                             ident16[:], is_transpose=True)

        ps_g1 = psA.tile([P, C // 4], F32, tag="pebm", name="pebm")
        for k in range(CH):
            nc.tensor.matmul(ps_g1[:], spT[:, k, :], Wg1_16[k][:],
                             start=(k == 0), stop=(k == CH - 1))
        g1 = work.tile([P, C // 4], F16, tag="m2g1", name="m2g1")
        nc.vector.tensor_tensor(out=g1[:], in0=ps_g1[:], in1=bg1_r[:], op=AL.add)
        nc.vector.tensor_scalar(out=g1[:], in0=g1[:], scalar1=0.0, scalar2=None,
                                op0=AL.max)
        ps_g1T = psA.tile([C // 4, P], F16, tag="hT", name="hT")
        nc.tensor.matmul(ps_g1T[:], g1[:], ident16[:],
                         is_transpose=True)
        g1T = work.tile([C // 4, P], F16, tag="m2g1T", name="m2g1T")
        nc.scalar.copy(g1T[:], ps_g1T[:])
        ps_g2 = psA.tile([P, 1], F32, tag="mm", name="mm")
        nc.tensor.matmul(ps_g2[:], g1T[:], Wg2_16[:], start=True, stop=True)
        ge = work.tile([P, 1], F32, tag="m2ge", name="m2ge")
        nc.scalar.activation(out=ge[:], in_=ps_g2[:], func=AF.Exp,
                             bias=nbg2[:], scale=neg1[:])
        nc.vector.tensor_scalar(out=ge[:], in0=ge[:], scalar1=1.0,
                                scalar2=None, op0=AL.add)
        gate = work.tile([P, 1], F32, tag="m2gate", name="m2gate")
        nc.vector.reciprocal(out=gate[:], in_=ge[:])
        nc.vector.tensor_scalar(out=gate[:], in0=gate[:], scalar1=0.2,
                                scalar2=None, op0=AL.mult)

        dmix = work.tile([P, C], F32, tag="m2dm", name="m2dm")
        nc.vector.tensor_tensor(out=dmix[:], in0=ps_en[:], in1=spat[:],
                                op=AL.subtract)
        outt = work.tile([P, C], F32, tag="m2out", name="m2out")
        nc.vector.scalar_tensor_tensor(out=outt[:], in0=dmix[:], scalar=gate[:],
                                       in1=spat[:], op0=AL.mult, op1=AL.add)
        nc.sync.dma_start(out=out_p.ap()[r0:r0 + P, :], in_=outt[:])

    ctx.close()


# ----------------------------------------------------------------------
# host entry point
# ----------------------------------------------------------------------
def _prep_inputs(inputs):
    f32 = np.float32
    feat = np.asarray(inputs["feat"], f32)
    coord = np.asarray(inputs["coord"], f32)
    ri = np.asarray(inputs["reference_index"]).astype(np.int32)
    idxc = np.maximum(ri, 0).astype(np.int32)
    idxraw_f = ri.astype(f32)

    Wq, Wk, Wv = (np.asarray(inputs[k], f32) for k in ("Wq", "Wk", "Wv"))
    Wqkv = np.ascontiguousarray(np.concatenate([Wq, Wk, Wv], axis=1))
    glw = np.asarray(inputs["glw"], f32)
    R = np.zeros((C, G), f32)
    R[np.arange(C), np.arange(C) // (C // G)] = glw
    Wp2 = np.asarray(inputs["Wp2"], f32)
    Wp2w = np.ascontiguousarray(Wp2 @ R)
    bp2 = np.asarray(inputs["bp2"], f32)
    bp2w = bp2 @ R

    D = _dct_matrix(C // G)
    F = _freq_filter(C // G)
    M = (D.T @ np.diag(F) @ D).astype(f32)
    BD = np.zeros((C, C), f32)
    for g in range(G):
        BD[g * 16:(g + 1) * 16, g * 16:(g + 1) * 16] = M

    Ww = np.asarray(inputs["Ww"], f32)
    WwBD = np.zeros((S * G, S * G), f32)
    for s in range(S):
        WwBD[s * G:(s + 1) * G, s * G:(s + 1) * G] = Ww
    bias_rows = np.zeros((8, C), f32)
    bias_rows[0] = np.asarray(inputs["bv"], f32) + bp2
    bias_rows[1] = np.asarray(inputs["bp1"], f32)
    bias_rows[2, :C // 4] = np.asarray(inputs["bg1"], f32)
    bias_rows[3, 0] = np.asarray(inputs["bg2"], f32).reshape(-1)[0]
    bias_rows[4, :G] = bp2w
    bias_rows[5, :G] = np.asarray(inputs["gw"], f32)
    bias_rows[6, :G] = np.asarray(inputs["betaw"], f32)

    qk_cm = np.concatenate([np.asarray(inputs[k], f32)
                            for k in ("gq", "betaq", "gk", "betak")]).reshape(-1, 1)
    p_cm = np.concatenate([np.asarray(inputs[k], f32)
                           for k in ("gp", "betap", "bp1")]).reshape(-1, 1)
    b_cm = np.concatenate([np.asarray(inputs[k], f32)
                           for k in ("bq", "bk")]).reshape(-1, 1)

    featT = np.ascontiguousarray(feat.T)
    coordT = np.ascontiguousarray(coord.T)

    shared = {
        "coord_full": coord,
        "Wqkv": Wqkv, "Wp1": np.asarray(inputs["Wp1"], f32), "Wp2": Wp2,
        "Wp2w": Wp2w, "Rglw": R, "WwBD": WwBD,
        "Wg1": np.asarray(inputs["Wg1"], f32),
        "Wg2": np.asarray(inputs["Wg2"], f32).reshape(C // 4, 1),
        "BDdct": BD, "bias_rows": bias_rows, "qk_cm": qk_cm, "p_cm": p_cm,
        "b_cm": b_cm,
    }
    in_maps = []
    for c in range(NCORES):
        sl = slice(c * NSH, (c + 1) * NSH)
        m = dict(shared)
        m["featT_sh"] = np.ascontiguousarray(featT[:, sl])
        m["coordT_sh"] = np.ascontiguousarray(coordT[:, sl])
        m["idxc"] = np.ascontiguousarray(idxc[sl])
        m["idxraw_f"] = np.ascontiguousarray(idxraw_f[sl])
        in_maps.append(m)
    return in_maps


def run(inputs, debug=False, trace=False):
    import os
    from concourse.bass_utils import run_bass_kernel_spmd
    key = ("nc", debug)
    if key not in _cache:
        _cache[key] = build_nc(debug=debug)
    nc = _cache[key]
    in_maps = _prep_inputs(inputs)
    res = run_bass_kernel_spmd(nc, in_maps, core_ids=list(range(NCORES)),
                               trace=trace,
                               tmpdir=os.environ.get("KTRACE_DIR"))
    return res


def kernel(**inputs):
    res = run(inputs)
    out = np.concatenate([res.results[c]["out"] for c in range(NCORES)], axis=0)
    return out.astype(np.float32)
